# revision 3
# baseline (speedup 1.0000x reference)
"""LorentzLinear forward on 8 Trainium2 NeuronCores.

Computes, for x [65536, 512], W [512, 512], b [512], scale []:
    y      = x @ W.T + b
    time   = sigmoid(y[:, :1]) * exp(scale) + 1.1
    xn     = y[:, 1:]
    denom  = clip(sum(xn * xn, -1, keepdims), 1e-8, None)
    out    = concat([time, xn * sqrt((time^2 - 1) / denom)], -1)

Sharding: data-parallel over rows — 8192 rows per core; W/b/scale replicated.

Per-core device program (SPMD, identical on all cores):
  - W.T is pre-laid-out on host as bf16 [512(i), 512(o)] and DMA'd once to SBUF.
  - For each [128, 512] row tile of x:
      PE-transpose (fp32, via identity) -> PSUM, DVE copy-cast to bf16 SBUF,
      4 accumulating bf16 matmuls against W.T -> y in PSUM [128 rows, 512],
      epilogue on DVE+ACT (fused square+row-reduce, sigmoid, per-row rescale
      fused into the PSUM->SBUF copy), DMA out.
"""

import math

import numpy as np

N, D = 65536, 512
N_CORES = 8
N_PER_CORE = N // N_CORES  # 8192
P = 128
N_TILES = N_PER_CORE // P  # 64
KC = D // P  # 4 contraction chunks

# "bf16": single bf16 matmul (fast; rel err ~1e-2 elementwise worst case)
# "bf16x3": split x and W into hi+lo bf16, 3 matmuls (rel err ~1e-5)
PRECISION = "bf16"

_program_cache = {}


def _build_program(with_bias: bool, precision: str):
    import concourse.bass as bass
    import concourse.tile as tile
    from concourse import bacc, mybir
    from concourse.masks import make_identity

    FT = mybir.ActivationFunctionType
    ALU = mybir.AluOpType
    f32 = mybir.dt.float32
    bf16 = mybir.dt.bfloat16

    nc = bacc.Bacc(num_devices=N_CORES)
    x_d = nc.dram_tensor("x", [N_PER_CORE, D], f32, kind="ExternalInput")
    wt_d = nc.dram_tensor("wt", [D, D], bf16, kind="ExternalInput")  # W.T [i, o]
    if precision == "bf16x3":
        wtlo_d = nc.dram_tensor("wtlo", [D, D], bf16, kind="ExternalInput")
    es_d = nc.dram_tensor("es", [P, 1], f32, kind="ExternalInput")  # exp(scale)
    if with_bias:
        b_d = nc.dram_tensor("b", [1, D], f32, kind="ExternalInput")
    out_d = nc.dram_tensor("out", [N_PER_CORE, D], f32, kind="ExternalOutput")

    with tile.TileContext(nc) as tc:
        with (
            tc.tile_pool(name="singles", bufs=1) as singles,
            tc.tile_pool(name="xin", bufs=3) as xin_pool,
            tc.tile_pool(name="xt", bufs=3) as xt_pool,
            tc.tile_pool(name="outp", bufs=3) as out_pool,
            tc.tile_pool(name="scratch", bufs=2) as scratch,
            tc.tile_pool(name="small", bufs=6) as small,
            tc.tile_pool(name="psum_t", bufs=2, space="PSUM") as psum_t,
            tc.tile_pool(name="psum_y", bufs=2, space="PSUM") as psum_y,
        ):
            wt_sb = singles.tile([P, KC, D], bf16)
            nc.sync.dma_start(
                out=wt_sb, in_=wt_d.ap().rearrange("(kc p) o -> p kc o", p=P)
            )
            if precision == "bf16x3":
                wtlo_sb = singles.tile([P, KC, D], bf16)
                nc.sync.dma_start(
                    out=wtlo_sb, in_=wtlo_d.ap().rearrange("(kc p) o -> p kc o", p=P)
                )
            es_sb = singles.tile([P, 1], f32)
            nc.sync.dma_start(out=es_sb, in_=es_d.ap())
            ident = singles.tile([P, P], f32)
            make_identity(nc, ident)
            if with_bias:
                b_sb = singles.tile([P, D], f32)
                b_ap = b_d.ap()
                b_bcast = bass.AP(
                    tensor=b_ap.tensor,
                    offset=b_ap.offset,
                    ap=[[0, P], b_ap.ap[1]],
                )
                nc.sync.dma_start(out=b_sb, in_=b_bcast)

            for t in range(N_TILES):
                x_sb = xin_pool.tile([P, D], f32)
                nc.sync.dma_start(out=x_sb, in_=x_d[t * P:(t + 1) * P, :])

                # Transpose x tile: [128 rows, 512 i] -> [128 i, kc, 128 rows]
                xt_ps = psum_t.tile([P, KC, P], f32)
                for k in range(KC):
                    nc.tensor.transpose(
                        xt_ps[:, k, :], x_sb[:, k * P:(k + 1) * P], ident
                    )
                xt_sb = xt_pool.tile([P, KC, P], bf16)
                nc.vector.tensor_copy(out=xt_sb, in_=xt_ps)
                if precision == "bf16x3":
                    # lo = round_bf16(x - hi): second bf16 word of the mantissa
                    xres_sb = xt_pool.tile([P, KC, P], f32, tag="xres")
                    nc.vector.tensor_tensor(
                        out=xres_sb, in0=xt_ps, in1=xt_sb, op=ALU.subtract
                    )
                    xtlo_sb = xt_pool.tile([P, KC, P], bf16, tag="xtlo")
                    nc.vector.tensor_copy(out=xtlo_sb, in_=xres_sb)

                y_ps = psum_y.tile([P, D], f32)
                if precision == "bf16x3":
                    n_mm = 3 * KC
                    i_mm = 0
                    for k in range(KC):
                        for lhs, rhs in (
                            (xt_sb, wt_sb),
                            (xt_sb, wtlo_sb),
                            (xtlo_sb, wt_sb),
                        ):
                            nc.tensor.matmul(
                                y_ps,
                                lhsT=lhs[:, k, :],
                                rhs=rhs[:, k, :],
                                start=(i_mm == 0),
                                stop=(i_mm == n_mm - 1),
                            )
                            i_mm += 1
                else:
                    for k in range(KC):
                        nc.tensor.matmul(
                            y_ps,
                            lhsT=xt_sb[:, k, :],
                            rhs=wt_sb[:, k, :],
                            start=(k == 0),
                            stop=(k == KC - 1),
                        )
                if with_bias:
                    nc.vector.tensor_add(y_ps, y_ps, b_sb)

                # Epilogue.
                # d = sum(y[:,1:]^2) per row: ACT Square with running-sum
                # accumulator (only one PSUM input allowed per instruction).
                sq_scr = scratch.tile([P, D - 1], f32)
                d_sb = small.tile([P, 1], f32, tag="d")
                nc.scalar.activation(
                    out=sq_scr,
                    in_=y_ps[:, 1:],
                    func=FT.Square,
                    accum_out=d_sb,
                )
                nc.vector.tensor_scalar_max(d_sb, d_sb, 1e-8)
                rd_sb = small.tile([P, 1], f32, tag="rd")
                nc.vector.reciprocal(rd_sb, d_sb)

                out_sb = out_pool.tile([P, D], f32)
                # time = sigmoid(y0) * es + 1.1, written straight into col 0
                tsig_sb = small.tile([P, 1], f32, tag="tsig")
                nc.scalar.activation(out=tsig_sb, in_=y_ps[:, 0:1], func=FT.Sigmoid)
                nc.scalar.activation(
                    out=out_sb[:, 0:1],
                    in_=tsig_sb,
                    func=FT.Copy,
                    scale=es_sb,
                    bias=1.1,
                )
                # s = (time^2 - 1) / d;  out[:,1:] = y[:,1:] * sqrt(s)
                u_sb = small.tile([P, 1], f32, tag="u")
                nc.vector.tensor_scalar(
                    out=u_sb,
                    in0=out_sb[:, 0:1],
                    scalar1=out_sb[:, 0:1],
                    scalar2=-1.0,
                    op0=ALU.mult,
                    op1=ALU.add,
                )
                s_sb = small.tile([P, 1], f32, tag="s")
                nc.vector.tensor_mul(s_sb, u_sb, rd_sb)
                sqs_sb = small.tile([P, 1], f32, tag="sqs")
                nc.scalar.activation(out=sqs_sb, in_=s_sb, func=FT.Sqrt)
                # out[:,1:] = y[:,1:] * sqrt(s): DVE per-partition-scalar mul,
                # doubles as the PSUM->SBUF copy.
                nc.vector.tensor_scalar_mul(out_sb[:, 1:], y_ps[:, 1:], sqs_sb)

                nc.sync.dma_start(out=out_d[t * P:(t + 1) * P, :], in_=out_sb)

    nc.compile()
    return nc


def _get_program(with_bias: bool, precision: str):
    key = (with_bias, precision)
    if key not in _program_cache:
        _program_cache[key] = _build_program(with_bias, precision)
    return _program_cache[key]


TRACE = False
LAST_RESULT = None  # BassKernelResults of the most recent run (for profiling)


def kernel(x, W, b, scale):
    import ml_dtypes
    from concourse.bass_utils import run_bass_kernel_spmd

    global LAST_RESULT

    x = np.asarray(x, dtype=np.float32)
    W = np.asarray(W, dtype=np.float32)
    b = np.asarray(b, dtype=np.float32)
    scale = np.asarray(scale, dtype=np.float32)
    assert x.shape == (N, D) and W.shape == (D, D) and b.shape == (D,)

    with_bias = bool(np.any(b != 0.0))
    nc = _get_program(with_bias, PRECISION)

    wt_f32 = np.ascontiguousarray(W.T)
    wt = wt_f32.astype(ml_dtypes.bfloat16)
    es = np.full((P, 1), np.exp(scale), dtype=np.float32)
    shared = {"wt": wt, "es": es}
    if PRECISION == "bf16x3":
        shared["wtlo"] = (wt_f32 - wt.astype(np.float32)).astype(ml_dtypes.bfloat16)
    if with_bias:
        shared["b"] = np.ascontiguousarray(b.reshape(1, D))

    in_maps = [
        {"x": np.ascontiguousarray(x[c * N_PER_CORE:(c + 1) * N_PER_CORE]), **shared}
        for c in range(N_CORES)
    ]
    res = run_bass_kernel_spmd(nc, in_maps, list(range(N_CORES)), trace=TRACE)
    LAST_RESULT = res
    return np.concatenate(
        [res.results[c]["out"] for c in range(N_CORES)], axis=0
    )


# revision 9
# speedup vs baseline: 1.8350x; 1.8350x over previous
"""LorentzLinear forward on 8 Trainium2 NeuronCores.

Computes, for x [65536, 512], W [512, 512], b [512], scale []:
    y      = x @ W.T + b
    time   = sigmoid(y[:, :1]) * exp(scale) + 1.1
    xn     = y[:, 1:]
    denom  = clip(sum(xn * xn, -1, keepdims), 1e-8, None)
    out    = concat([time, xn * sqrt((time^2 - 1) / denom)], -1)

Sharding: data-parallel over rows — 8192 rows per core; W/b/scale replicated.

Per-core device program (SPMD, identical on all cores):
  - W.T is pre-laid-out on host as bf16 [512(i), 512(o)] and DMA'd once to SBUF.
  - For each [128, 512] row tile of x:
      PE-transpose (fp32, via identity) -> PSUM, DVE copy-cast to bf16 SBUF,
      4 accumulating bf16 matmuls against W.T -> y in PSUM [128 rows, 512],
      epilogue on DVE+ACT (fused square+row-reduce, sigmoid, per-row rescale
      fused into the PSUM->SBUF copy), DMA out.
"""

import math

import numpy as np

N, D = 65536, 512
N_CORES = 8
N_PER_CORE = N // N_CORES  # 8192
P = 128
N_TILES = N_PER_CORE // P  # 64
KC = D // P  # 4 contraction chunks

# "bf16": single bf16 matmul (fast; rel err ~1e-2 elementwise worst case)
# "bf16x3": split x and W into hi+lo bf16, 3 matmuls (rel err ~1e-5)
PRECISION = "bf16"

_program_cache = {}


def _build_program(with_bias: bool, precision: str):
    import concourse.bass as bass
    import concourse.tile as tile
    from concourse import bacc, mybir
    from concourse.masks import make_identity

    FT = mybir.ActivationFunctionType
    ALU = mybir.AluOpType
    f32 = mybir.dt.float32
    bf16 = mybir.dt.bfloat16

    nc = bacc.Bacc(num_devices=N_CORES)
    x_d = nc.dram_tensor("x", [N_PER_CORE, D], f32, kind="ExternalInput")
    wt_d = nc.dram_tensor("wt", [D, D], bf16, kind="ExternalInput")  # W.T [i, o]
    if precision == "bf16x3":
        wtlo_d = nc.dram_tensor("wtlo", [D, D], bf16, kind="ExternalInput")
    es_d = nc.dram_tensor("es", [P, 1], f32, kind="ExternalInput")  # exp(scale)
    if with_bias:
        b_d = nc.dram_tensor("b", [1, D], f32, kind="ExternalInput")
    out_d = nc.dram_tensor("out", [N_PER_CORE, D], f32, kind="ExternalOutput")

    # Row tiles are [128, 512]; super-tiles pack 2 row tiles per DMA; groups of
    # G=4 row tiles share one batched sigmoid->sqrt epilogue so the ACT engine
    # switches activation-table sets twice per group instead of per tile.
    SROWS = 2  # row tiles per super tile (one DMA each way)
    G = 4  # row tiles per epilogue group (PSUM-bank limited)

    with tile.TileContext(nc) as tc:
        with (
            tc.tile_pool(name="singles", bufs=1) as singles,
            tc.tile_pool(name="xin", bufs=3) as xin_pool,
            tc.tile_pool(name="xt", bufs=3) as xt_pool,
            tc.tile_pool(name="outp", bufs=4) as out_pool,
            tc.tile_pool(name="small", bufs=3) as small,
            tc.tile_pool(name="psum_t", bufs=2, space="PSUM") as psum_t,
            tc.tile_pool(name="psum_y", bufs=6, space="PSUM") as psum_y,
        ):
            wt_sb = singles.tile([P, KC, D], bf16)
            nc.sync.dma_start(
                out=wt_sb, in_=wt_d.ap().rearrange("(kc p) o -> p kc o", p=P)
            )
            if precision == "bf16x3":
                wtlo_sb = singles.tile([P, KC, D], bf16)
                nc.sync.dma_start(
                    out=wtlo_sb, in_=wtlo_d.ap().rearrange("(kc p) o -> p kc o", p=P)
                )
            es_sb = singles.tile([P, 1], f32)
            nc.sync.dma_start(out=es_sb, in_=es_d.ap())
            ident = singles.tile([P, P], f32)
            make_identity(nc, ident)
            if with_bias:
                b_sb = singles.tile([P, D], f32)
                b_ap = b_d.ap()
                b_bcast = bass.AP(
                    tensor=b_ap.tensor,
                    offset=b_ap.offset,
                    ap=[[0, P], b_ap.ap[1]],
                )
                nc.sync.dma_start(out=b_sb, in_=b_bcast)

            NSUPER = N_TILES // SROWS  # 32
            group_y = []  # per row tile in the current group: y PSUM tile
            group_out = []  # matching SBUF output slice [P, D]
            group_dmas = []  # (dram_ap, out_sb) flushed at group end
            sg = dg = None

            for st in range(NSUPER):
                r0 = st * SROWS * P
                x_sb = xin_pool.tile([P, SROWS, D], f32)
                nc.sync.dma_start(
                    out=x_sb,
                    in_=x_d[r0:r0 + SROWS * P, :].rearrange(
                        "(s p) d -> p s d", p=P
                    ),
                )
                out_sb = out_pool.tile([P, SROWS, D], f32)

                for s in range(SROWS):
                    ti = st * SROWS + s  # global row-tile index
                    gi = ti % G  # index within epilogue group
                    if gi == 0:
                        sg = small.tile([P, G], f32, tag="sg")  # sigmoid(y0)
                        dg = small.tile([P, G], f32, tag="dg")  # sumsq/(D-1)

                    # Transpose x row tile: [128 rows, 512 i] -> [128 i, kc, 128 r]
                    xt_ps = psum_t.tile([P, KC, P], f32)
                    for k in range(KC):
                        nc.tensor.transpose(
                            xt_ps[:, k, :], x_sb[:, s, k * P:(k + 1) * P], ident
                        )
                    xt_sb = xt_pool.tile([P, KC, P], bf16)
                    nc.vector.tensor_copy(out=xt_sb, in_=xt_ps)
                    if precision == "bf16x3":
                        # lo = round_bf16(x - hi): next 8 mantissa bits
                        xres_sb = xt_pool.tile([P, KC, P], f32, tag="xres")
                        nc.vector.tensor_tensor(
                            out=xres_sb, in0=xt_ps, in1=xt_sb, op=ALU.subtract
                        )
                        xtlo_sb = xt_pool.tile([P, KC, P], bf16, tag="xtlo")
                        nc.vector.tensor_copy(out=xtlo_sb, in_=xres_sb)

                    y_ps = psum_y.tile([P, D], f32)
                    if precision == "bf16x3":
                        mms = [
                            (k, lhs, rhs)
                            for k in range(KC)
                            for lhs, rhs in (
                                (xt_sb, wt_sb),
                                (xt_sb, wtlo_sb),
                                (xtlo_sb, wt_sb),
                            )
                        ]
                    else:
                        mms = [(k, xt_sb, wt_sb) for k in range(KC)]
                    for i_mm, (k, lhs, rhs) in enumerate(mms):
                        nc.tensor.matmul(
                            y_ps,
                            lhsT=lhs[:, k, :],
                            rhs=rhs[:, k, :],
                            start=(i_mm == 0),
                            stop=(i_mm == len(mms) - 1),
                        )
                    if with_bias:
                        nc.vector.tensor_add(y_ps, y_ps, b_sb)

                    # Per-tile epilogue inputs (engines chosen to stay within
                    # one ACT table set: Sigmoid/Copy both live in set 2).
                    nc.scalar.activation(
                        out=sg[:, gi:gi + 1], in_=y_ps[:, 0:1], func=FT.Sigmoid
                    )
                    # sumsq via bn stats: sum(y^2) = n*(var + mean^2)
                    stats = small.tile([P, 6], f32, tag="stats")
                    nc.vector.bn_stats(out=stats, in_=y_ps[:, 1:])
                    mv = small.tile([P, 2], f32, tag="mv")
                    nc.vector.bn_aggr(out=mv, in_=stats)
                    nc.gpsimd.tensor_scalar(
                        out=dg[:, gi:gi + 1],
                        in0=mv[:, 0:1],
                        scalar1=mv[:, 0:1],
                        scalar2=mv[:, 1:2],
                        op0=ALU.mult,
                        op1=ALU.add,
                    )
                    group_y.append(y_ps)
                    group_out.append(out_sb[:, s, :])
                    if s == 0:
                        group_dmas.append(
                            (
                                out_d[r0:r0 + SROWS * P, :].rearrange(
                                    "(s p) d -> p s d", p=P
                                ),
                                out_sb,
                            )
                        )

                    if gi == G - 1:
                        # Group epilogue: d'=sumsq/(D-1); s=(t^2-1)/((D-1)*d')
                        nc.gpsimd.tensor_scalar_max(dg, dg, 1e-8 / (D - 1))
                        rd = small.tile([P, G], f32, tag="rd")
                        nc.vector.reciprocal(rd, dg)
                        tg = small.tile([P, G], f32, tag="tg")
                        nc.gpsimd.tensor_scalar(
                            out=tg,
                            in0=sg,
                            scalar1=es_sb,
                            scalar2=1.1,
                            op0=ALU.mult,
                            op1=ALU.add,
                        )
                        ug = small.tile([P, G], f32, tag="ug")
                        nc.gpsimd.tensor_tensor(out=ug, in0=tg, in1=tg, op=ALU.mult)
                        nc.gpsimd.tensor_scalar_add(ug, ug, -1.0)
                        s4 = small.tile([P, G], f32, tag="s4")
                        nc.gpsimd.tensor_tensor(out=s4, in0=ug, in1=rd, op=ALU.mult)
                        sqs = small.tile([P, G], f32, tag="sqs")
                        nc.scalar.activation(
                            out=sqs, in_=s4, func=FT.Sqrt, scale=1.0 / (D - 1)
                        )
                        for i in range(G):
                            nc.scalar.activation(
                                out=group_out[i][:, 1:],
                                in_=group_y[i][:, 1:],
                                func=FT.Copy,
                                scale=sqs[:, i:i + 1],
                            )
                            nc.gpsimd.tensor_copy(
                                out=group_out[i][:, 0:1], in_=tg[:, i:i + 1]
                            )
                        group_y.clear()
                        group_out.clear()
                        for dram_ap, sb in group_dmas:
                            nc.sync.dma_start(out=dram_ap, in_=sb)
                        group_dmas.clear()

            assert not group_y and not group_dmas

    nc.compile()
    return nc


def _get_program(with_bias: bool, precision: str):
    key = (with_bias, precision)
    if key not in _program_cache:
        _program_cache[key] = _build_program(with_bias, precision)
    return _program_cache[key]


TRACE = False
LAST_RESULT = None  # BassKernelResults of the most recent run (for profiling)


def kernel(x, W, b, scale):
    import ml_dtypes
    from concourse.bass_utils import run_bass_kernel_spmd

    global LAST_RESULT

    x = np.asarray(x, dtype=np.float32)
    W = np.asarray(W, dtype=np.float32)
    b = np.asarray(b, dtype=np.float32)
    scale = np.asarray(scale, dtype=np.float32)
    assert x.shape == (N, D) and W.shape == (D, D) and b.shape == (D,)

    with_bias = bool(np.any(b != 0.0))
    nc = _get_program(with_bias, PRECISION)

    wt_f32 = np.ascontiguousarray(W.T)
    wt = wt_f32.astype(ml_dtypes.bfloat16)
    es = np.full((P, 1), np.exp(scale), dtype=np.float32)
    shared = {"wt": wt, "es": es}
    if PRECISION == "bf16x3":
        shared["wtlo"] = (wt_f32 - wt.astype(np.float32)).astype(ml_dtypes.bfloat16)
    if with_bias:
        shared["b"] = np.ascontiguousarray(b.reshape(1, D))

    in_maps = [
        {"x": np.ascontiguousarray(x[c * N_PER_CORE:(c + 1) * N_PER_CORE]), **shared}
        for c in range(N_CORES)
    ]
    res = run_bass_kernel_spmd(nc, in_maps, list(range(N_CORES)), trace=TRACE)
    LAST_RESULT = res
    return np.concatenate(
        [res.results[c]["out"] for c in range(N_CORES)], axis=0
    )


# revision 10
# speedup vs baseline: 1.9245x; 1.0488x over previous
"""LorentzLinear forward on 8 Trainium2 NeuronCores.

Computes, for x [65536, 512], W [512, 512], b [512], scale []:
    y      = x @ W.T + b
    time   = sigmoid(y[:, :1]) * exp(scale) + 1.1
    xn     = y[:, 1:]
    denom  = clip(sum(xn * xn, -1, keepdims), 1e-8, None)
    out    = concat([time, xn * sqrt((time^2 - 1) / denom)], -1)

Sharding: data-parallel over rows — 8192 rows per core; W/b/scale replicated.

Device strategy (SPMD, identical program on all cores):
  - The matmul runs in bf16 (fp32 PSUM accumulation). x and W.T are cast to
    bf16 on the host (identical RNE rounding to a device-side cast) so the
    contraction-major x tiles can be loaded with hardware DMA-transpose
    (2-byte dtypes only) — no TensorE transpose pass, and half the input DMA.
  - Per 1024-row block: 4 transposing DMAs produce x.T [128(i), 4(kc), 1024(r)]
    in SBUF; per 128-row tile: 4 accumulating matmuls vs resident W.T.
  - Epilogue per tile: ACT sigmoid on y[:,0]; DVE bn_stats/bn_aggr give
    sum(y^2) = n*(var+mean^2); GpSimd does the small algebra; groups of G=4
    tiles share one batched ACT sqrt so the ACT activation-table set switches
    only twice per group; DVE per-row-scalar multiply writes the scaled
    output (doubling as the PSUM->SBUF copy).
"""

import math

import numpy as np

N, D = 65536, 512
N_CORES = 8
N_PER_CORE = N // N_CORES  # 8192
P = 128
KC = D // P  # 4 contraction chunks
R_BLK = 1024  # rows per DMA-transpose block
N_BLK = N_PER_CORE // R_BLK  # 8
TPB = R_BLK // P  # 8 row tiles per block

# "bf16": single bf16 matmul (rel err ~1e-3)
# "bf16x3": x and W split into hi+lo bf16, 3 matmuls (rel err ~1e-5)
PRECISION = "bf16"

_program_cache = {}


def _build_program(with_bias: bool, precision: str):
    import concourse.bass as bass
    import concourse.tile as tile
    from concourse import bacc, mybir

    FT = mybir.ActivationFunctionType
    ALU = mybir.AluOpType
    f32 = mybir.dt.float32
    bf16 = mybir.dt.bfloat16

    nc = bacc.Bacc(num_devices=N_CORES)
    xb_d = nc.dram_tensor("xb", [N_PER_CORE, D], bf16, kind="ExternalInput")
    wt_d = nc.dram_tensor("wt", [D, D], bf16, kind="ExternalInput")  # W.T [i, o]
    if precision == "bf16x3":
        xlo_d = nc.dram_tensor("xlo", [N_PER_CORE, D], bf16, kind="ExternalInput")
        wtlo_d = nc.dram_tensor("wtlo", [D, D], bf16, kind="ExternalInput")
    es_d = nc.dram_tensor("es", [P, 1], f32, kind="ExternalInput")  # exp(scale)
    if with_bias:
        b_d = nc.dram_tensor("b", [1, D], f32, kind="ExternalInput")
    out_d = nc.dram_tensor("out", [N_PER_CORE, D], f32, kind="ExternalOutput")

    SROWS = 2  # row tiles per output super tile (one store DMA each)
    G = 4  # row tiles per batched epilogue group

    with tile.TileContext(nc) as tc:
        with (
            tc.tile_pool(name="singles", bufs=1) as singles,
            tc.tile_pool(name="xtp", bufs=2) as xtp_pool,
            tc.tile_pool(name="outp", bufs=4) as out_pool,
            tc.tile_pool(name="small", bufs=3) as small,
            tc.tile_pool(name="psum_y", bufs=7, space="PSUM") as psum_y,
        ):
            wt_sb = singles.tile([P, KC, D], bf16)
            nc.sync.dma_start(
                out=wt_sb, in_=wt_d.ap().rearrange("(kc p) o -> p kc o", p=P)
            )
            if precision == "bf16x3":
                wtlo_sb = singles.tile([P, KC, D], bf16)
                nc.sync.dma_start(
                    out=wtlo_sb, in_=wtlo_d.ap().rearrange("(kc p) o -> p kc o", p=P)
                )
            es_sb = singles.tile([P, 1], f32)
            nc.sync.dma_start(out=es_sb, in_=es_d.ap())
            if with_bias:
                b_ap = b_d.ap()
                b_sb = singles.tile([P, D], f32)
                nc.sync.dma_start(
                    out=b_sb,
                    in_=bass.AP(
                        tensor=b_ap.tensor, offset=b_ap.offset, ap=[[0, P], b_ap.ap[1]]
                    ),
                )

            group_y = []  # per row tile in current group: y PSUM tile
            group_out = []  # matching SBUF output slice [P, D]
            group_dmas = []  # (dram_ap, out_sb) flushed at group end
            sg = dg = out_sb = None

            for b in range(N_BLK):
                rb = b * R_BLK
                xt_blk = xtp_pool.tile([P, KC, R_BLK], bf16)
                for k in range(KC):
                    nc.sync.dma_start_transpose(
                        xt_blk[:, k, :],
                        xb_d[rb:rb + R_BLK, k * P:(k + 1) * P],
                    )
                if precision == "bf16x3":
                    xtlo_blk = xtp_pool.tile([P, KC, R_BLK], bf16, tag="xtlo")
                    for k in range(KC):
                        nc.sync.dma_start_transpose(
                            xtlo_blk[:, k, :],
                            xlo_d[rb:rb + R_BLK, k * P:(k + 1) * P],
                        )

                for j in range(TPB):
                    ti = b * TPB + j  # global row-tile index
                    gi = ti % G
                    s = ti % SROWS
                    if s == 0:
                        r0 = rb + j * P
                        out_sb = out_pool.tile([P, SROWS, D], f32)
                        group_dmas.append(
                            (
                                out_d[r0:r0 + SROWS * P, :].rearrange(
                                    "(s p) d -> p s d", p=P
                                ),
                                out_sb,
                            )
                        )
                    if gi == 0:
                        sg = small.tile([P, G], f32, tag="sg")  # sigmoid(y0)
                        dg = small.tile([P, G], f32, tag="dg")  # sumsq/(D-1)

                    y_ps = psum_y.tile([P, D], f32)
                    if precision == "bf16x3":
                        mms = [
                            (k, lhs, rhs)
                            for k in range(KC)
                            for lhs, rhs in (
                                (xt_blk, wt_sb),
                                (xt_blk, wtlo_sb),
                                (xtlo_blk, wt_sb),
                            )
                        ]
                    else:
                        mms = [(k, xt_blk, wt_sb) for k in range(KC)]
                    for i_mm, (k, lhs, rhs) in enumerate(mms):
                        nc.tensor.matmul(
                            y_ps,
                            lhsT=lhs[:, k, j * P:(j + 1) * P],
                            rhs=rhs[:, k, :],
                            start=(i_mm == 0),
                            stop=(i_mm == len(mms) - 1),
                        )
                    if with_bias:
                        nc.vector.tensor_add(y_ps, y_ps, b_sb)

                    # Per-tile epilogue inputs. Sigmoid/Copy share ACT table
                    # set 2; the only set switch is the per-group Sqrt.
                    nc.scalar.activation(
                        out=sg[:, gi:gi + 1], in_=y_ps[:, 0:1], func=FT.Sigmoid
                    )
                    # sumsq via bn stats: sum(y^2) = n*(var + mean^2)
                    stats = small.tile([P, 6], f32, tag="stats")
                    nc.vector.bn_stats(out=stats, in_=y_ps[:, 1:])
                    mv = small.tile([P, 2], f32, tag="mv")
                    nc.vector.bn_aggr(out=mv, in_=stats)
                    nc.gpsimd.tensor_scalar(
                        out=dg[:, gi:gi + 1],
                        in0=mv[:, 0:1],
                        scalar1=mv[:, 0:1],
                        scalar2=mv[:, 1:2],
                        op0=ALU.mult,
                        op1=ALU.add,
                    )
                    group_y.append(y_ps)
                    group_out.append(out_sb[:, s, :])

                    if gi == G - 1:
                        # d' = sumsq/(D-1); s = (t^2-1)/((D-1)*d')
                        nc.gpsimd.tensor_scalar_max(dg, dg, 1e-8 / (D - 1))
                        rd = small.tile([P, G], f32, tag="rd")
                        nc.vector.reciprocal(rd, dg)
                        tg = small.tile([P, G], f32, tag="tg")
                        nc.gpsimd.tensor_scalar(
                            out=tg,
                            in0=sg,
                            scalar1=es_sb,
                            scalar2=1.1,
                            op0=ALU.mult,
                            op1=ALU.add,
                        )
                        ug = small.tile([P, G], f32, tag="ug")
                        nc.gpsimd.tensor_tensor(out=ug, in0=tg, in1=tg, op=ALU.mult)
                        nc.gpsimd.tensor_scalar_add(ug, ug, -1.0)
                        s4 = small.tile([P, G], f32, tag="s4")
                        nc.gpsimd.tensor_tensor(out=s4, in0=ug, in1=rd, op=ALU.mult)
                        sqs = small.tile([P, G], f32, tag="sqs")
                        nc.scalar.activation(
                            out=sqs, in_=s4, func=FT.Sqrt, scale=1.0 / (D - 1)
                        )
                        for i in range(G):
                            nc.vector.tensor_scalar_mul(
                                group_out[i][:, 1:], group_y[i][:, 1:], sqs[:, i:i + 1]
                            )
                            nc.gpsimd.tensor_copy(
                                out=group_out[i][:, 0:1], in_=tg[:, i:i + 1]
                            )
                        group_y.clear()
                        group_out.clear()
                        for dram_ap, sb in group_dmas:
                            nc.sync.dma_start(out=dram_ap, in_=sb)
                        group_dmas.clear()

            assert not group_y and not group_dmas

    nc.compile()
    return nc


def _get_program(with_bias: bool, precision: str):
    key = (with_bias, precision)
    if key not in _program_cache:
        _program_cache[key] = _build_program(with_bias, precision)
    return _program_cache[key]


TRACE = False
LAST_RESULT = None  # BassKernelResults of the most recent run (for profiling)


def kernel(x, W, b, scale):
    import ml_dtypes
    from concourse.bass_utils import run_bass_kernel_spmd

    global LAST_RESULT

    x = np.asarray(x, dtype=np.float32)
    W = np.asarray(W, dtype=np.float32)
    b = np.asarray(b, dtype=np.float32)
    scale = np.asarray(scale, dtype=np.float32)
    assert x.shape == (N, D) and W.shape == (D, D) and b.shape == (D,)

    with_bias = bool(np.any(b != 0.0))
    nc = _get_program(with_bias, PRECISION)

    xb = x.astype(ml_dtypes.bfloat16)
    wt_f32 = np.ascontiguousarray(W.T)
    wt = wt_f32.astype(ml_dtypes.bfloat16)
    es = np.full((P, 1), np.exp(scale), dtype=np.float32)
    shared = {"wt": wt, "es": es}
    if PRECISION == "bf16x3":
        shared["wtlo"] = (wt_f32 - wt.astype(np.float32)).astype(ml_dtypes.bfloat16)
        xlo = (x - xb.astype(np.float32)).astype(ml_dtypes.bfloat16)
    if with_bias:
        shared["b"] = np.ascontiguousarray(b.reshape(1, D))

    in_maps = []
    for c in range(N_CORES):
        rows = slice(c * N_PER_CORE, (c + 1) * N_PER_CORE)
        m = {"xb": np.ascontiguousarray(xb[rows]), **shared}
        if PRECISION == "bf16x3":
            m["xlo"] = np.ascontiguousarray(xlo[rows])
        in_maps.append(m)
    res = run_bass_kernel_spmd(nc, in_maps, list(range(N_CORES)), trace=TRACE)
    LAST_RESULT = res
    return np.concatenate(
        [res.results[c]["out"] for c in range(N_CORES)], axis=0
    )


# revision 13
# speedup vs baseline: 1.9332x; 1.0045x over previous
"""LorentzLinear forward on 8 Trainium2 NeuronCores.

Computes, for x [65536, 512], W [512, 512], b [512], scale []:
    y      = x @ W.T + b
    time   = sigmoid(y[:, :1]) * exp(scale) + 1.1
    xn     = y[:, 1:]
    denom  = clip(sum(xn * xn, -1, keepdims), 1e-8, None)
    out    = concat([time, xn * sqrt((time^2 - 1) / denom)], -1)

Sharding: data-parallel over rows — 8192 rows per core; W/b/scale replicated.

Device strategy (SPMD, identical program on all cores):
  - The matmul runs in bf16 (fp32 PSUM accumulation). x and W.T are cast to
    bf16 on the host (identical RNE rounding to a device-side cast) so the
    contraction-major x tiles can be loaded with hardware DMA-transpose
    (2-byte dtypes only) — no TensorE transpose pass, and half the input DMA.
  - Per 1024-row block: 4 transposing DMAs produce x.T [128(i), 4(kc), 1024(r)]
    in SBUF; per 128-row tile: 4 accumulating matmuls vs resident W.T.
  - Epilogue per tile: ACT sigmoid on y[:,0]; DVE bn_stats/bn_aggr give
    sum(y^2) = n*(var+mean^2); GpSimd does the small algebra; groups of G=4
    tiles share one batched ACT sqrt so the ACT activation-table set switches
    only twice per group; DVE per-row-scalar multiply writes the scaled
    output (doubling as the PSUM->SBUF copy).
"""

import math

import numpy as np

N, D = 65536, 512
N_CORES = 8
N_PER_CORE = N // N_CORES  # 8192
P = 128
KC = D // P  # 4 contraction chunks
R_BLK = 1024  # rows per DMA-transpose block
N_BLK = N_PER_CORE // R_BLK  # 8
TPB = R_BLK // P  # 8 row tiles per block

# "bf16": single bf16 matmul (rel err ~1e-3)
# "bf16x3": x and W split into hi+lo bf16, 3 matmuls (rel err ~1e-5)
PRECISION = "bf16"

_program_cache = {}


def _build_program(with_bias: bool, precision: str):
    import concourse.bass as bass
    import concourse.tile as tile
    from concourse import bacc, mybir

    FT = mybir.ActivationFunctionType
    ALU = mybir.AluOpType
    f32 = mybir.dt.float32
    bf16 = mybir.dt.bfloat16

    nc = bacc.Bacc(num_devices=N_CORES)
    xb_d = nc.dram_tensor("xb", [N_PER_CORE, D], bf16, kind="ExternalInput")
    wt_d = nc.dram_tensor("wt", [D, D], bf16, kind="ExternalInput")  # W.T [i, o]
    if precision == "bf16x3":
        xlo_d = nc.dram_tensor("xlo", [N_PER_CORE, D], bf16, kind="ExternalInput")
        wtlo_d = nc.dram_tensor("wtlo", [D, D], bf16, kind="ExternalInput")
    es_d = nc.dram_tensor("es", [P, 1], f32, kind="ExternalInput")  # exp(scale)
    if with_bias:
        b_d = nc.dram_tensor("b", [1, D], f32, kind="ExternalInput")
    out_d = nc.dram_tensor("out", [N_PER_CORE, D], f32, kind="ExternalOutput")

    SROWS = 2  # row tiles per output super tile (one store DMA each)
    G = 4  # row tiles per batched epilogue group

    with tile.TileContext(nc) as tc:
        with (
            tc.tile_pool(name="singles", bufs=1) as singles,
            tc.tile_pool(name="xtp", bufs=3) as xtp_pool,
            tc.tile_pool(name="outp", bufs=6) as out_pool,
            tc.tile_pool(name="small", bufs=4) as small,
            tc.tile_pool(name="psum_y", bufs=8, space="PSUM") as psum_y,
        ):
            wt_sb = singles.tile([P, KC, D], bf16)
            nc.sync.dma_start(
                out=wt_sb, in_=wt_d.ap().rearrange("(kc p) o -> p kc o", p=P)
            )
            if precision == "bf16x3":
                wtlo_sb = singles.tile([P, KC, D], bf16)
                nc.sync.dma_start(
                    out=wtlo_sb, in_=wtlo_d.ap().rearrange("(kc p) o -> p kc o", p=P)
                )
            es_sb = singles.tile([P, 1], f32)
            nc.sync.dma_start(out=es_sb, in_=es_d.ap())
            if with_bias:
                b_ap = b_d.ap()
                b_sb = singles.tile([P, D], f32)
                nc.sync.dma_start(
                    out=b_sb,
                    in_=bass.AP(
                        tensor=b_ap.tensor, offset=b_ap.offset, ap=[[0, P], b_ap.ap[1]]
                    ),
                )

            group_y = []  # per row tile in current group: y PSUM tile
            group_out = []  # matching SBUF output slice [P, D]
            group_dmas = []  # (dram_ap, out_sb) flushed at group end
            pending_groups = []  # deferred finales (one group deep)
            sg = dg = out_sb = None

            def _emit_group_finale(s4, tg, g_y, g_out, g_dmas):
                sqs = small.tile([P, G], f32, tag="sqs")
                nc.scalar.activation(
                    out=sqs, in_=s4, func=FT.Sqrt, scale=1.0 / (D - 1)
                )
                for i in range(G):
                    if i % 2 == 0:
                        nc.scalar.activation(
                            out=g_out[i][:, 1:],
                            in_=g_y[i][:, 1:],
                            func=FT.Copy,
                            scale=sqs[:, i:i + 1],
                        )
                    else:
                        nc.vector.tensor_scalar_mul(
                            g_out[i][:, 1:], g_y[i][:, 1:], sqs[:, i:i + 1]
                        )
                    nc.gpsimd.tensor_copy(
                        out=g_out[i][:, 0:1], in_=tg[:, i:i + 1]
                    )
                for dram_ap, sb in g_dmas:
                    nc.gpsimd.dma_start(out=dram_ap, in_=sb)

            for b in range(N_BLK):
                rb = b * R_BLK
                xt_blk = xtp_pool.tile([P, KC, R_BLK], bf16)
                for k in range(KC):
                    nc.sync.dma_start_transpose(
                        xt_blk[:, k, :],
                        xb_d[rb:rb + R_BLK, k * P:(k + 1) * P],
                    )
                if precision == "bf16x3":
                    xtlo_blk = xtp_pool.tile([P, KC, R_BLK], bf16, tag="xtlo")
                    for k in range(KC):
                        nc.sync.dma_start_transpose(
                            xtlo_blk[:, k, :],
                            xlo_d[rb:rb + R_BLK, k * P:(k + 1) * P],
                        )

                for j in range(TPB):
                    ti = b * TPB + j  # global row-tile index
                    gi = ti % G
                    s = ti % SROWS
                    if s == 0:
                        r0 = rb + j * P
                        out_sb = out_pool.tile([P, SROWS, D], f32)
                        group_dmas.append(
                            (
                                out_d[r0:r0 + SROWS * P, :].rearrange(
                                    "(s p) d -> p s d", p=P
                                ),
                                out_sb,
                            )
                        )
                    if gi == 0:
                        sg = small.tile([P, G], f32, tag="sg")  # sigmoid(y0)
                        dg = small.tile([P, G], f32, tag="dg")  # sumsq/(D-1)

                    y_ps = psum_y.tile([P, D], f32)
                    if precision == "bf16x3":
                        mms = [
                            (k, lhs, rhs)
                            for k in range(KC)
                            for lhs, rhs in (
                                (xt_blk, wt_sb),
                                (xt_blk, wtlo_sb),
                                (xtlo_blk, wt_sb),
                            )
                        ]
                    else:
                        mms = [(k, xt_blk, wt_sb) for k in range(KC)]
                    for i_mm, (k, lhs, rhs) in enumerate(mms):
                        nc.tensor.matmul(
                            y_ps,
                            lhsT=lhs[:, k, j * P:(j + 1) * P],
                            rhs=rhs[:, k, :],
                            start=(i_mm == 0),
                            stop=(i_mm == len(mms) - 1),
                        )
                    if with_bias:
                        nc.vector.tensor_add(y_ps, y_ps, b_sb)

                    # Per-tile epilogue inputs. Sigmoid/Copy share ACT table
                    # set 2; the only set switch is the per-group Sqrt.
                    nc.scalar.activation(
                        out=sg[:, gi:gi + 1], in_=y_ps[:, 0:1], func=FT.Sigmoid
                    )
                    # sumsq via bn stats: sum(y^2) = n*(var + mean^2)
                    stats = small.tile([P, 6], f32, tag="stats")
                    nc.vector.bn_stats(out=stats, in_=y_ps[:, 1:])
                    mv = small.tile([P, 2], f32, tag="mv")
                    nc.vector.bn_aggr(out=mv, in_=stats)
                    nc.gpsimd.tensor_scalar(
                        out=dg[:, gi:gi + 1],
                        in0=mv[:, 0:1],
                        scalar1=mv[:, 0:1],
                        scalar2=mv[:, 1:2],
                        op0=ALU.mult,
                        op1=ALU.add,
                    )
                    group_y.append(y_ps)
                    group_out.append(out_sb[:, s, :])

                    if gi == G - 1:
                        # d' = sumsq/(D-1); s = (t^2-1)/((D-1)*d')
                        nc.gpsimd.tensor_scalar_max(dg, dg, 1e-8 / (D - 1))
                        rd = small.tile([P, G], f32, tag="rd")
                        nc.vector.reciprocal(rd, dg)
                        tg = small.tile([P, G], f32, tag="tg")
                        nc.gpsimd.tensor_scalar(
                            out=tg,
                            in0=sg,
                            scalar1=es_sb,
                            scalar2=1.1,
                            op0=ALU.mult,
                            op1=ALU.add,
                        )
                        ug = small.tile([P, G], f32, tag="ug")
                        nc.gpsimd.tensor_tensor(out=ug, in0=tg, in1=tg, op=ALU.mult)
                        nc.gpsimd.tensor_scalar_add(ug, ug, -1.0)
                        s4 = small.tile([P, G], f32, tag="s4")
                        nc.gpsimd.tensor_tensor(out=s4, in0=ug, in1=rd, op=ALU.mult)
                        # Defer the sqrt + output scaling of this group until
                        # the NEXT group's sigmoids are queued: the ACT queue
                        # then runs sig x8, sqrt x2, ... so the activation
                        # table switches twice per TWO groups instead of two
                        # per group. Needs 2*G PSUM banks (all 8).
                        pending_groups.append(
                            (s4, tg, list(group_y), list(group_out), list(group_dmas))
                        )
                        group_y.clear()
                        group_out.clear()
                        group_dmas.clear()
                        if len(pending_groups) > 1:
                            _emit_group_finale(*pending_groups.pop(0))

            while pending_groups:
                _emit_group_finale(*pending_groups.pop(0))
            assert not group_y and not group_dmas

    nc.compile()
    return nc


def _get_program(with_bias: bool, precision: str):
    key = (with_bias, precision)
    if key not in _program_cache:
        _program_cache[key] = _build_program(with_bias, precision)
    return _program_cache[key]


TRACE = False
LAST_RESULT = None  # BassKernelResults of the most recent run (for profiling)


def kernel(x, W, b, scale):
    import ml_dtypes
    from concourse.bass_utils import run_bass_kernel_spmd

    global LAST_RESULT

    x = np.asarray(x, dtype=np.float32)
    W = np.asarray(W, dtype=np.float32)
    b = np.asarray(b, dtype=np.float32)
    scale = np.asarray(scale, dtype=np.float32)
    assert x.shape == (N, D) and W.shape == (D, D) and b.shape == (D,)

    with_bias = bool(np.any(b != 0.0))
    nc = _get_program(with_bias, PRECISION)

    xb = x.astype(ml_dtypes.bfloat16)
    wt_f32 = np.ascontiguousarray(W.T)
    wt = wt_f32.astype(ml_dtypes.bfloat16)
    es = np.full((P, 1), np.exp(scale), dtype=np.float32)
    shared = {"wt": wt, "es": es}
    if PRECISION == "bf16x3":
        shared["wtlo"] = (wt_f32 - wt.astype(np.float32)).astype(ml_dtypes.bfloat16)
        xlo = (x - xb.astype(np.float32)).astype(ml_dtypes.bfloat16)
    if with_bias:
        shared["b"] = np.ascontiguousarray(b.reshape(1, D))

    in_maps = []
    for c in range(N_CORES):
        rows = slice(c * N_PER_CORE, (c + 1) * N_PER_CORE)
        m = {"xb": np.ascontiguousarray(xb[rows]), **shared}
        if PRECISION == "bf16x3":
            m["xlo"] = np.ascontiguousarray(xlo[rows])
        in_maps.append(m)
    res = run_bass_kernel_spmd(nc, in_maps, list(range(N_CORES)), trace=TRACE)
    LAST_RESULT = res
    return np.concatenate(
        [res.results[c]["out"] for c in range(N_CORES)], axis=0
    )


# revision 16
# speedup vs baseline: 2.1261x; 1.0998x over previous
"""LorentzLinear forward on 8 Trainium2 NeuronCores.

Computes, for x [65536, 512], W [512, 512], b [512], scale []:
    y      = x @ W.T + b
    time   = sigmoid(y[:, :1]) * exp(scale) + 1.1
    xn     = y[:, 1:]
    denom  = clip(sum(xn * xn, -1, keepdims), 1e-8, None)
    out    = concat([time, xn * sqrt((time^2 - 1) / denom)], -1)

Sharding: data-parallel over rows — 8192 rows per core; W/b/scale replicated.

Device strategy (SPMD, identical program on all cores):
  - The matmul runs in bf16 (fp32 PSUM accumulation). x and W.T are cast to
    bf16 on the host (identical RNE rounding to a device-side cast) so the
    contraction-major x tiles can be loaded with hardware DMA-transpose
    (2-byte dtypes only) — no TensorE transpose pass, and half the input DMA.
  - Per 1024-row block: 4 transposing DMAs produce x.T [128(i), 4(kc), 1024(r)]
    in SBUF; per 128-row tile: 4 accumulating matmuls vs resident W.T.
  - Epilogue per tile: ACT sigmoid on y[:,0]; DVE bn_stats/bn_aggr give
    sum(y^2) = n*(var+mean^2); GpSimd does the small algebra; groups of G=4
    tiles share one batched ACT sqrt so the ACT activation-table set switches
    only twice per group; DVE per-row-scalar multiply writes the scaled
    output (doubling as the PSUM->SBUF copy).
"""

import math

import numpy as np

N, D = 65536, 512
N_CORES = 8
N_PER_CORE = N // N_CORES  # 8192
P = 128
KC = D // P  # 4 contraction chunks
R_BLK = 1024  # rows per DMA-transpose block
N_BLK = N_PER_CORE // R_BLK  # 8
TPB = R_BLK // P  # 8 row tiles per block

# "bf16": single bf16 matmul (rel err ~1e-3)
# "bf16x3": x and W split into hi+lo bf16, 3 matmuls (rel err ~1e-5)
PRECISION = "bf16"

_program_cache = {}


def _build_program(with_bias: bool, precision: str):
    import concourse.bass as bass
    import concourse.tile as tile
    from concourse import bacc, mybir

    FT = mybir.ActivationFunctionType
    ALU = mybir.AluOpType
    f32 = mybir.dt.float32
    bf16 = mybir.dt.bfloat16

    nc = bacc.Bacc(num_devices=N_CORES)
    xb_d = nc.dram_tensor("xb", [N_PER_CORE, D], bf16, kind="ExternalInput")
    wt_d = nc.dram_tensor("wt", [D, D], bf16, kind="ExternalInput")  # W.T [i, o]
    if precision == "bf16x3":
        xlo_d = nc.dram_tensor("xlo", [N_PER_CORE, D], bf16, kind="ExternalInput")
        wtlo_d = nc.dram_tensor("wtlo", [D, D], bf16, kind="ExternalInput")
    es_d = nc.dram_tensor("es", [P, 1], f32, kind="ExternalInput")  # exp(scale)
    if with_bias:
        b_d = nc.dram_tensor("b", [1, D], f32, kind="ExternalInput")
    out_d = nc.dram_tensor("out", [N_PER_CORE, D], f32, kind="ExternalOutput")

    SROWS = 2  # row tiles per output super tile (one store DMA each)
    G = 4  # row tiles per batched epilogue group

    with tile.TileContext(nc) as tc:
        with (
            tc.tile_pool(name="singles", bufs=1) as singles,
            tc.tile_pool(name="xtp", bufs=3) as xtp_pool,
            tc.tile_pool(name="outp", bufs=6) as out_pool,
            tc.tile_pool(name="small", bufs=4) as small,
            tc.tile_pool(name="psum_y", bufs=8, space="PSUM") as psum_y,
        ):
            wt_sb = singles.tile([P, KC, D], bf16)
            nc.sync.dma_start(
                out=wt_sb, in_=wt_d.ap().rearrange("(kc p) o -> p kc o", p=P)
            )
            if precision == "bf16x3":
                wtlo_sb = singles.tile([P, KC, D], bf16)
                nc.sync.dma_start(
                    out=wtlo_sb, in_=wtlo_d.ap().rearrange("(kc p) o -> p kc o", p=P)
                )
            es_sb = singles.tile([P, 1], f32)
            nc.sync.dma_start(out=es_sb, in_=es_d.ap())
            if with_bias:
                b_ap = b_d.ap()
                b_sb = singles.tile([P, D], f32)
                nc.sync.dma_start(
                    out=b_sb,
                    in_=bass.AP(
                        tensor=b_ap.tensor, offset=b_ap.offset, ap=[[0, P], b_ap.ap[1]]
                    ),
                )

            group_y = []  # per row tile in current group: y PSUM tile
            group_out = []  # matching SBUF output slice [P, D]
            group_dmas = []  # (dram_ap, out_sb) flushed at group end
            sg = dg = out_sb = None
            i32 = mybir.dt.int32
            RSQRT_MAGIC = 0x5F3759DF

            for b in range(N_BLK):
                rb = b * R_BLK
                xt_blk = xtp_pool.tile([P, KC, R_BLK], bf16)
                for k in range(KC):
                    nc.sync.dma_start_transpose(
                        xt_blk[:, k, :],
                        xb_d[rb:rb + R_BLK, k * P:(k + 1) * P],
                    )
                if precision == "bf16x3":
                    xtlo_blk = xtp_pool.tile([P, KC, R_BLK], bf16, tag="xtlo")
                    for k in range(KC):
                        nc.sync.dma_start_transpose(
                            xtlo_blk[:, k, :],
                            xlo_d[rb:rb + R_BLK, k * P:(k + 1) * P],
                        )

                for j in range(TPB):
                    ti = b * TPB + j  # global row-tile index
                    gi = ti % G
                    s = ti % SROWS
                    if s == 0:
                        r0 = rb + j * P
                        out_sb = out_pool.tile([P, SROWS, D], f32)
                        group_dmas.append(
                            (
                                out_d[r0:r0 + SROWS * P, :].rearrange(
                                    "(s p) d -> p s d", p=P
                                ),
                                out_sb,
                            )
                        )
                    if gi == 0:
                        sg = small.tile([P, G], f32, tag="sg")  # sigmoid(y0)
                        dg = small.tile([P, G], f32, tag="dg")  # sumsq/(D-1)

                    y_ps = psum_y.tile([P, D], f32)
                    if precision == "bf16x3":
                        mms = [
                            (k, lhs, rhs)
                            for k in range(KC)
                            for lhs, rhs in (
                                (xt_blk, wt_sb),
                                (xt_blk, wtlo_sb),
                                (xtlo_blk, wt_sb),
                            )
                        ]
                    else:
                        mms = [(k, xt_blk, wt_sb) for k in range(KC)]
                    for i_mm, (k, lhs, rhs) in enumerate(mms):
                        nc.tensor.matmul(
                            y_ps,
                            lhsT=lhs[:, k, j * P:(j + 1) * P],
                            rhs=rhs[:, k, :],
                            start=(i_mm == 0),
                            stop=(i_mm == len(mms) - 1),
                        )
                    if with_bias:
                        nc.vector.tensor_add(y_ps, y_ps, b_sb)

                    # Per-tile epilogue inputs. Sigmoid/Copy share ACT table
                    # set 2; the only set switch is the per-group Sqrt.
                    nc.scalar.activation(
                        out=sg[:, gi:gi + 1], in_=y_ps[:, 0:1], func=FT.Sigmoid
                    )
                    # sumsq via bn stats: sum(y^2) = n*(var + mean^2)
                    stats = small.tile([P, 6], f32, tag="stats")
                    nc.vector.bn_stats(out=stats, in_=y_ps[:, 1:])
                    mv = small.tile([P, 2], f32, tag="mv")
                    nc.vector.bn_aggr(out=mv, in_=stats)
                    nc.gpsimd.tensor_scalar(
                        out=dg[:, gi:gi + 1],
                        in0=mv[:, 0:1],
                        scalar1=mv[:, 0:1],
                        scalar2=mv[:, 1:2],
                        op0=ALU.mult,
                        op1=ALU.add,
                    )
                    group_y.append(y_ps)
                    group_out.append(out_sb[:, s, :])

                    if gi == G - 1:
                        # Group epilogue. out[:,1:] = y*sqrt(u/dsum) with
                        # u = t^2-1, dsum = max((D-1)*d', 1e-8), computed as
                        # u*rsqrt(u*dsum) via a quake-seed Newton iteration —
                        # no ACT Sqrt, so the ACT engine never switches
                        # activation-table sets (sigmoid set stays resident).
                        dq = small.tile([P, G], f32, tag="dq")
                        nc.gpsimd.tensor_scalar(
                            out=dq,
                            in0=dg,
                            scalar1=float(D - 1),
                            scalar2=1e-8,
                            op0=ALU.mult,
                            op1=ALU.max,
                        )
                        tg = small.tile([P, G], f32, tag="tg")
                        nc.gpsimd.tensor_scalar(
                            out=tg,
                            in0=sg,
                            scalar1=es_sb,
                            scalar2=1.1,
                            op0=ALU.mult,
                            op1=ALU.add,
                        )
                        ug = small.tile([P, G], f32, tag="ug")
                        nc.gpsimd.tensor_tensor(out=ug, in0=tg, in1=tg, op=ALU.mult)
                        nc.gpsimd.tensor_scalar_add(ug, ug, -1.0)
                        zg = small.tile([P, G], f32, tag="zg")
                        nc.vector.tensor_tensor(out=zg, in0=ug, in1=dq, op=ALU.mult)
                        # rsqrt seed: r = bits_to_f32(MAGIC - (f32_bits(z) >> 1))
                        jt = small.tile([P, G], i32, tag="jt")
                        nc.vector.tensor_scalar(
                            out=jt,
                            in0=zg.bitcast(i32),
                            scalar1=1,
                            scalar2=None,
                            op0=ALU.logical_shift_right,
                        )
                        nc.vector.tensor_scalar(
                            out=jt,
                            in0=jt,
                            scalar1=RSQRT_MAGIC,
                            scalar2=-1,
                            op0=ALU.subtract,
                            op1=ALU.mult,
                        )
                        r = jt.bitcast(f32)
                        n_iters = 3 if precision == "bf16x3" else 2
                        for _ in range(n_iters):
                            ra = small.tile([P, G], f32, tag="ra")
                            nc.vector.tensor_tensor(out=ra, in0=r, in1=r, op=ALU.mult)
                            nc.vector.tensor_tensor(out=ra, in0=ra, in1=zg, op=ALU.mult)
                            nc.vector.tensor_scalar(
                                out=ra,
                                in0=ra,
                                scalar1=-0.5,
                                scalar2=1.5,
                                op0=ALU.mult,
                                op1=ALU.add,
                            )
                            rn = small.tile([P, G], f32, tag="rn")
                            nc.vector.tensor_tensor(out=rn, in0=r, in1=ra, op=ALU.mult)
                            r = rn
                        sqs = small.tile([P, G], f32, tag="sqs")
                        nc.vector.tensor_tensor(out=sqs, in0=ug, in1=r, op=ALU.mult)
                        for i in range(G):
                            nc.scalar.activation(
                                out=group_out[i][:, 1:],
                                in_=group_y[i][:, 1:],
                                func=FT.Copy,
                                scale=sqs[:, i:i + 1],
                            )
                            nc.gpsimd.tensor_copy(
                                out=group_out[i][:, 0:1], in_=tg[:, i:i + 1]
                            )
                        group_y.clear()
                        group_out.clear()
                        for dram_ap, sb in group_dmas:
                            nc.sync.dma_start(out=dram_ap, in_=sb)
                        group_dmas.clear()

            assert not group_y and not group_dmas

    nc.compile()
    return nc


def _get_program(with_bias: bool, precision: str):
    key = (with_bias, precision)
    if key not in _program_cache:
        _program_cache[key] = _build_program(with_bias, precision)
    return _program_cache[key]


TRACE = False
LAST_RESULT = None  # BassKernelResults of the most recent run (for profiling)


def kernel(x, W, b, scale):
    import ml_dtypes
    from concourse.bass_utils import run_bass_kernel_spmd

    global LAST_RESULT

    x = np.asarray(x, dtype=np.float32)
    W = np.asarray(W, dtype=np.float32)
    b = np.asarray(b, dtype=np.float32)
    scale = np.asarray(scale, dtype=np.float32)
    assert x.shape == (N, D) and W.shape == (D, D) and b.shape == (D,)

    with_bias = bool(np.any(b != 0.0))
    nc = _get_program(with_bias, PRECISION)

    xb = x.astype(ml_dtypes.bfloat16)
    wt_f32 = np.ascontiguousarray(W.T)
    wt = wt_f32.astype(ml_dtypes.bfloat16)
    es = np.full((P, 1), np.exp(scale), dtype=np.float32)
    shared = {"wt": wt, "es": es}
    if PRECISION == "bf16x3":
        shared["wtlo"] = (wt_f32 - wt.astype(np.float32)).astype(ml_dtypes.bfloat16)
        xlo = (x - xb.astype(np.float32)).astype(ml_dtypes.bfloat16)
    if with_bias:
        shared["b"] = np.ascontiguousarray(b.reshape(1, D))

    in_maps = []
    for c in range(N_CORES):
        rows = slice(c * N_PER_CORE, (c + 1) * N_PER_CORE)
        m = {"xb": np.ascontiguousarray(xb[rows]), **shared}
        if PRECISION == "bf16x3":
            m["xlo"] = np.ascontiguousarray(xlo[rows])
        in_maps.append(m)
    res = run_bass_kernel_spmd(nc, in_maps, list(range(N_CORES)), trace=TRACE)
    LAST_RESULT = res
    return np.concatenate(
        [res.results[c]["out"] for c in range(N_CORES)], axis=0
    )
